# revision 4
# baseline (speedup 1.0000x reference)
"""Bi-directional minGRU Trainium2 kernel.

Full-input contract: kernel(**inputs) takes the unsharded numpy inputs from
reference.setup_inputs() and returns the full (B, L, 1) float32 output.

Sharding: data-parallel over batch B=32 across 8 NeuronCores (4 sequences
per core), parameters replicated. Per core, feature-on-partition /
time-on-free layout:

  rr     : [t_enc1(64) ; x(2) ; ones(1)] = 67 rows x L. The ones row (sent
           inside the x DMA) carries every bias (input-proj, gate,
           time-enc-2, head) folded into the composed weights host-side, so
           no activation needs a per-partition bias and one ACT instruction
           can span both 128-channel halves of H=256.
  t_enc1 : relu(w1*t+b1) in one fused ACT instruction per sequence
           (per-partition scale/bias), prepared just-in-time one sequence
           ahead; the first sequence uses DVE so ACT reaches the gates
           sooner. t is broadcast to 64 partitions by a stride-0 DMA.
  gates  : per (dir, func, 1024-col time block): one [128, 2048] PSUM slot
           holds [h0|h1]; sigmoid(-pre) -> a = 1-z and tanh(pre) -> h~ on
           ACT, 2048 wide, written through a 3-level AP into a [128, 2L+2]
           tile with two permanent zero columns.
  b      : -b = (a-1)*h~ in one DVE pass, in place over the h~ tile.
  scan   : ONE DVE tensor_tensor_scan per (seq, dir) covering both halves;
           the zero columns reset the recurrence between the halves (a=0,
           b=0 -> state=0) and the fwd/bwd AP alignment realizes the
           one-step output shift (the reference stores the pre-update
           state) with no extra instructions. The backward direction runs
           through negative-stride APs.
  head   : [h_f0;h_f1;h_b0;h_b1;rr] @ g1 accumulated on PE in 1024-col
           blocks interleaved between gate GEMMs; relu+bias-fold consume
           split half/half between ACT and DVE (both near-saturated).
  g2     : k=128 -> 1 batched per sequence PAIR with column-masked copies
           of gh_w2 accumulating into a [2, L] PSUM tile (matmul outputs
           must start at partition 0), ACT copies to SBUF and one DMA
           stores each pair as soon as its heads finish. gh_b2 on host.

Measured on hardware via repeat-slope (see bench): ~95-100 us/core vs
~110-117 us for the previous version; engine-busy floor is ~80 us
(ACT sigmoid/tanh 58 us; DVE scans 55 us + elementwise).
"""

import time

import numpy as np
import ml_dtypes

import concourse.bass as bass
import concourse.mybir as mybir
import concourse.tile as tile
from concourse.vector_clock import ScopedClock, VectorClock
from concourse.bass_utils import run_bass_kernel_spmd

# ---------------------------------------------------------------------------
# Workaround for a walrus codegen limit in this toolchain: the TileContext
# tail drain carries one sync-wait per live proc sem, but this walrus build
# rejects >2 sync waits on a Drain (CTRL_NO_STRUCT template). Re-emit the tail
# with the waits split across single-wait NOPs on the sync engine (same-engine
# program order preserves the semantics), followed by a wait-free drain.
# ---------------------------------------------------------------------------


def _patched_drain_and_barrier(self, tick_clock, wait_clock):
    nc = self.nc
    vals = list(tick_clock.global_clock)
    n = len(vals)
    for i, v in enumerate(vals):
        if v > 0:
            partial = [0] * n
            partial[i] = v
            nop = nc.sync.nop()
            wait_clock.add_sem_waits(nop.ins, ScopedClock({None: VectorClock(partial)}))
    nc.sync.drain()
    nc.all_engine_barrier()
    assert self.sems is not None
    popped = nc._tile_sem_poison_stack.pop()
    assert popped is self._sem_poison
    nc.clear_and_free_semaphores(list(self.sems.allocated().values()))
    nc.all_engine_barrier()


tile.TileContext._drain_and_barrier = _patched_drain_and_barrier


def _spill_excess_waits(nc, maxw=1):
    """Split instructions carrying more than `maxw` sem waits: the excess
    waits move onto NoOps inserted just before, on the same engine (same-
    engine program order keeps the semantics identical)."""
    for bb in nc.m.functions[0].blocks:
        new = []
        for inst in bb.instructions:
            si = inst.sync_info
            if si is not None and si.on_wait is not None and len(si.on_wait) > maxw:
                waits = list(si.on_wait)
                excess, keep = waits[:-maxw], waits[-maxw:]
                for j, w in enumerate(excess):
                    nop = mybir.InstNoOp(
                        name=f"{inst.name}_ws{j}",
                        engine=inst.engine,
                        ins=[],
                        outs=[],
                        sync_info=mybir.SyncInfo(on_wait=[w], on_update=[]),
                    )
                    nc.register_instruction(nop)
                    new.append(nop)
                si.on_wait = keep
            new.append(inst)
        if len(new) != len(bb.instructions):
            _replace_block_instructions(bb, new)


def _replace_block_instructions(bb, new):
    try:
        bb.instructions = new
    except Exception:
        while len(bb.instructions):
            bb.instructions.pop()
        for inst in new:
            bb.add_instruction(inst)

# ---------------------------------------------------------------------------

B, L, H, TE = 32, 2048, 256, 64
NCORES = 8
BS = B // NCORES           # sequences per core
HH = 128                   # gauss head hidden
IN_AUG = TE + 2 + 1        # rr rows: t_enc1(64) + x(2) + ones(1)
F32 = mybir.dt.float32

DT = mybir.dt.bfloat16     # matmul/activation storage dtype
NP_DT = ml_dtypes.bfloat16

TB = 1024                  # gate time-block (one [128, 2048] slot = [h0|h1])
MM = 512                   # matmul moving-operand chunk


def _rev(t, cols, ncols):
    """Reversed-free-dim view of tile AP t over columns [cols, cols+ncols)."""
    return bass.AP(
        tensor=t.tensor,
        offset=t.offset + cols + ncols - 1,
        ap=[list(t.ap[0]), [-1, ncols]],
    )


def _halves_view(t, tb, d):
    """[128, 2, TB] view of a [128, 2L+2] gate tile: both halves' block tb.

    Column layout (fwd): [Z | h0 time 0:L | Z | h1 time 0:L]
    Column layout (bwd): [h0 time 0:L | Z | h1 time 0:L | Z]
    where Z columns hold permanent zeros so one fused scan over the whole
    tile resets its state (a=0, b=0) between the halves; the fwd/bwd offset
    difference also realizes the one-step output shift for free."""
    return bass.AP(
        tensor=t.tensor,
        offset=t.offset + tb * TB + (1 if d == "f" else 0),
        ap=[list(t.ap[0]), [L + 1, 2], [1, TB]],
    )


def _build_nc(bs=BS, repeats=1, relu_dve=4, gtb=1024, g2_early=False,
              jit_te=True, tbc_sync=True, te_late=True, g2_pair=True,
              stt_pool=0, stt_mode="dve"):
    nc = bass.Bass("TRN2", target_bir_lowering=False, debug=False, num_devices=NCORES)

    d_xT = nc.dram_tensor("xT", [bs, 3, L], DT, kind="ExternalInput")
    d_t = nc.dram_tensor("t", [bs, L], DT, kind="ExternalInput")
    # composed gate weights in the R basis: cols [z_f | h_f | z_b | h_b],
    # each [67, 256]; every bias folded into the ones row (66)
    d_wk = nc.dram_tensor("wk", [IN_AUG, 4 * H], DT, kind="ExternalInput")
    # head weights: g1 row-blocks packed on cols; g1te carries gh_b1's fold
    d_g1k = nc.dram_tensor("g1k", [128, 4 * HH], DT, kind="ExternalInput")
    d_g1te = nc.dram_tensor("g1te", [IN_AUG, HH], DT, kind="ExternalInput")
    # g2 as bs column-masked copies: block bi has gh_w2 in col bi, else 0
    d_g2m = nc.dram_tensor("g2m", [HH, 2 * bs], DT, kind="ExternalInput")
    d_tw = nc.dram_tensor("tw", [TE, 2], F32, kind="ExternalInput")
    d_out = nc.dram_tensor("out", [bs, L], F32, kind="ExternalOutput")

    with tile.TileContext(nc) as tc:
        with (
            tc.tile_pool(name="wpool", bufs=1) as wp,
            tc.tile_pool(name="mpool", bufs=2) as mp,
            tc.tile_pool(name="hpool", bufs=2) as hp,
            tc.tile_pool(name="psum", bufs=2, space="PSUM") as pp,
        ):
            # ---- replicated weights: 5 DMAs, ordered by first use ----
            s_tw = wp.tile([TE, 2], F32, tag="tw", name="tw")
            nc.scalar.dma_start(out=s_tw, in_=d_tw[:, :])
            s_wk = wp.tile([IN_AUG, 4 * H], DT, tag="wk", name="wk")
            nc.scalar.dma_start(out=s_wk, in_=d_wk[:, :])
            s_g1k = wp.tile([128, 4 * HH], DT, tag="g1k", name="g1k")
            nc.scalar.dma_start(out=s_g1k, in_=d_g1k[:, :])
            s_g1te = wp.tile([IN_AUG, HH], DT, tag="g1te", name="g1te")
            nc.scalar.dma_start(out=s_g1te, in_=d_g1te[:, :])
            s_g2m = wp.tile([HH, 2 * bs], DT, tag="g2m", name="g2m")
            nc.scalar.dma_start(out=s_g2m, in_=d_g2m[:, :])

            def gate_w(d, func, half):
                off = (0 if d == "f" else 2 * H) + (0 if func == "z" else H)
                return s_wk[:, off + 128 * half:off + 128 * (half + 1)]

            for r in range(repeats):
                SW = 2 * gtb                   # PSUM slot columns
                W = 2 * L + 2                  # gate tile width incl. Z cols
                rts = []
                rrs = [None] * bs

                tbcs = [None] * bs

                def prep_dma(bi):
                    rr = mp.tile([IN_AUG, L], DT, tag="rr", name="rr", bufs=bs)
                    nc.sync.dma_start(out=rr[TE:TE + 3, :], in_=d_xT[bi])
                    t_bc = mp.tile([TE, L], DT, tag="t_bc", name="t_bc")
                    trow = d_t[bi:bi + 1, :]
                    (nc.sync if tbc_sync else nc.gpsimd).dma_start(
                        out=t_bc,
                        in_=bass.AP(tensor=trow.tensor, offset=trow.offset,
                                    ap=[[0, TE], list(trow.ap[-1])]))
                    return rr, t_bc

                def te_act(bi):
                    rr, t_bc = rrs[bi], tbcs[bi]
                    # relu(w1*t + b1): one fused ACT instruction (per-
                    # partition scale/bias). For the first sequence use the
                    # (otherwise idle-at-start) DVE instead so ACT can get to
                    # the gates sooner. Pool relu/max lowerings are
                    # catastrophically slow software loops - never there.
                    if bi == 0:
                        nc.vector.tensor_scalar(out=rr[0:TE, :], in0=t_bc,
                                                scalar1=s_tw[:, 0:1],
                                                scalar2=s_tw[:, 1:2],
                                                op0=mybir.AluOpType.mult,
                                                op1=mybir.AluOpType.add)
                        nc.vector.tensor_scalar(out=rr[0:TE, :],
                                                in0=rr[0:TE, :],
                                                scalar1=0.0, scalar2=None,
                                                op0=mybir.AluOpType.max)
                    else:
                        nc.scalar.activation(out=rr[0:TE, :], in_=t_bc,
                                             func=mybir.ActivationFunctionType.Relu,
                                             bias=s_tw[:, 1:2], scale=s_tw[:, 0:1])

                def prep_rr(bi):
                    rrs[bi], tbcs[bi] = prep_dma(bi)
                    te_act(bi)

                def zcols(d):
                    return (0, L + 1) if d == "f" else (L, 2 * L + 1)

                def gate_tile(tag, d, first):
                    """[128, W] gate tile; zero the two Z columns once per
                    ring buffer (they self-maintain afterwards)."""
                    t = mp.tile([128, W], DT, tag=tag, name=tag)
                    if first:
                        for zc in zcols(d):
                            nc.gpsimd.memset(t[:, zc:zc + 1], 0.0)
                    return t

                def halves_view(t, tb, d):
                    return bass.AP(
                        tensor=t.tensor,
                        offset=t.offset + tb * gtb + (1 if d == "f" else 0),
                        ap=[list(t.ap[0]), [L + 1, 2], [1, gtb]],
                    )

                def gate_gemm(rr, d, func, dst):
                    """One [128, SW] PSUM slot = [h0|h1] of a gtb-col time
                    block -> one ACT instruction per block."""
                    for tb in range(L // gtb):
                        ps = pp.tile([128, SW], F32, tag="ps", name="ps")
                        for half in range(2):
                            w = gate_w(d, func, half)
                            for ch in range(gtb // MM):
                                c0 = tb * gtb + ch * MM
                                nc.tensor.matmul(
                                    ps[:, half * gtb + ch * MM:
                                       half * gtb + (ch + 1) * MM],
                                    lhsT=w, rhs=rr[:, c0:c0 + MM],
                                    start=True, stop=True)
                        if func == "z":
                            nc.scalar.activation(
                                out=halves_view(dst, tb, d), in_=ps,
                                func=mybir.ActivationFunctionType.Sigmoid,
                                scale=-1.0)
                        else:
                            nc.scalar.activation(
                                out=halves_view(dst, tb, d), in_=ps,
                                func=mybir.ActivationFunctionType.Tanh)

                ndve = 8 - stt_pool
                dve_sids = ({3, 7} if stt_pool == 6 else
                            set() if ndve == 0 else
                            {int((i + 0.5) * 8 / ndve) for i in range(ndve)})

                def emit_scan(bi, d, at, ht):
                    """-b = (a-1)*h~, then ONE fused shifted scan over both
                    halves; the Z columns reset the recurrence between them
                    and give the one-step output shift free. Most -b passes
                    run as two Pool tensor_tensor instructions (throughput
                    trade: Pool is otherwise idle; its Z columns compute to
                    zero on their own)."""
                    sid = 2 * bi + (0 if d == "f" else 1)
                    if sid in dve_sids or stt_pool == 0:
                        bt = ht
                        nc.vector.scalar_tensor_tensor(
                            out=ht, in0=at, scalar=1.0, in1=ht,
                            op0=mybir.AluOpType.subtract,
                            op1=mybir.AluOpType.mult)
                    elif stt_mode == "pool":
                        bt = mp.tile([128, W], DT, tag=f"bt_{d}",
                                     name=f"bt_{d}")
                        nc.gpsimd.tensor_tensor(out=bt, in0=at, in1=ht,
                                                op=mybir.AluOpType.mult)
                        nc.gpsimd.tensor_tensor(out=bt, in0=bt, in1=ht,
                                                op=mybir.AluOpType.subtract)
                    else:   # hybrid: Pool multiply, DVE subtract
                        bt = mp.tile([128, W], DT, tag=f"bt_{d}",
                                     name=f"bt_{d}")
                        nc.gpsimd.tensor_tensor(out=bt, in0=at, in1=ht,
                                                op=mybir.AluOpType.mult)
                        nc.vector.tensor_tensor(out=bt, in0=bt, in1=ht,
                                                op=mybir.AluOpType.subtract)
                    hv = hp.tile([128, W], DT, tag=f"hv{d}", name=f"hv{d}")
                    if d == "f":
                        nc.vector.tensor_tensor_scan(
                            out=hv[:, 0:W], data0=at[:, 0:W],
                            data1=bt[:, 0:W], initial=0.0,
                            op0=mybir.AluOpType.mult,
                            op1=mybir.AluOpType.subtract)
                    else:
                        nc.vector.tensor_tensor_scan(
                            out=_rev(hv, 0, W - 1),
                            data0=_rev(at, 1, W - 1),
                            data1=_rev(bt, 1, W - 1), initial=0.0,
                            op0=mybir.AluOpType.mult,
                            op1=mybir.AluOpType.subtract)
                    return hv


                def head_work(bi, hvs):
                    """g1 GEMM + relu in 1024-col blocks (deferred PE
                    closures); after each odd sequence, the pair's g2 runs,
                    its rows are copied out and stored (pipelined tail)."""
                    rr = rrs[bi]
                    rt = mp.tile([HH, L], DT, tag="rt", name="rt", bufs=3)
                    ktiles = [(s_g1k[:, 0:HH], hvs["f"], 0),
                              (s_g1k[:, HH:2 * HH], hvs["f"], L + 1),
                              (s_g1k[:, 2 * HH:3 * HH], hvs["b"], 0),
                              (s_g1k[:, 3 * HH:4 * HH], hvs["b"], L + 1),
                              (s_g1te, rr, 0)]
                    HB = 1024
                    chunks = []
                    for blk in range(L // HB):
                        def chunk(blk=blk, rt=rt):
                            ps = pp.tile([128, SW], F32, tag="ps", name="ps")
                            for ki, (w, rhs, c) in enumerate(ktiles):
                                for ch in range(HB // MM):
                                    c0 = c + blk * HB + ch * MM
                                    nc.tensor.matmul(
                                        ps[:, ch * MM:(ch + 1) * MM], lhsT=w,
                                        rhs=rhs[:, c0:c0 + MM],
                                        start=(ki == 0), stop=(ki == 4))
                            if 2 * bi + blk < relu_dve:
                                nc.vector.tensor_scalar(
                                    out=rt[:, blk * HB:(blk + 1) * HB],
                                    in0=ps[:, 0:HB], scalar1=0.0, scalar2=None,
                                    op0=mybir.AluOpType.max)
                            else:
                                nc.scalar.activation(
                                    out=rt[:, blk * HB:(blk + 1) * HB],
                                    in_=ps[:, 0:HB],
                                    func=mybir.ActivationFunctionType.Relu)
                        chunks.append(chunk)
                    rts.append(rt)

                    if g2_pair and bi % 2 == 1:
                        def g2_pair_fn(bi=bi):
                            pr = bi // 2
                            ps = pp.tile([2, L], F32, tag="ps", name="psg2")
                            for seg in range(L // MM):
                                for j in range(2):
                                    nc.tensor.matmul(
                                        ps[:, seg * MM:(seg + 1) * MM],
                                        lhsT=s_g2m[:, 2 * (2 * pr + j):
                                                   2 * (2 * pr + j) + 2],
                                        rhs=rts[2 * pr + j][:,
                                                            seg * MM:(seg + 1) * MM],
                                        start=(j == 0), stop=(j == 1))
                            orow = mp.tile([2, L], F32, tag="orow",
                                           name="orow")
                            nc.scalar.copy(out=orow, in_=ps)
                            nc.sync.dma_start(
                                out=d_out[2 * pr:2 * pr + 2, :],
                                in_=orow)
                        chunks.append(g2_pair_fn)
                    return chunks

                pending = []

                def drain(k):
                    for _ in range(k):
                        if pending:
                            pending.pop(0)()

                prep_rr(0)
                for bi in range(bs):
                    rr = rrs[bi]
                    first = r == 0 and bi < 2
                    at_f = gate_tile("at_f", "f", first)
                    at_b = gate_tile("at_b", "b", first)
                    ht_f = gate_tile("ht_f", "f", first)
                    ht_b = gate_tile("ht_b", "b", first)
                    gate_gemm(rr, "f", "z", at_f)
                    if jit_te and bi + 1 < bs:
                        if te_late:
                            rrs[bi + 1], tbcs[bi + 1] = prep_dma(bi + 1)
                        else:
                            prep_rr(bi + 1)
                    gate_gemm(rr, "f", "h", ht_f)
                    if jit_te and te_late and bi + 1 < bs:
                        te_act(bi + 1)
                    drain(1)
                    hvs = {}
                    hvs["f"] = emit_scan(bi, "f", at_f, ht_f)
                    gate_gemm(rr, "b", "z", at_b)
                    drain(1)
                    gate_gemm(rr, "b", "h", ht_b)
                    hvs["b"] = emit_scan(bi, "b", at_b, ht_b)
                    drain(4)
                    pending = head_work(bi, hvs)
                drain(4)

                if not g2_pair:
                    for pr in range(bs // 2):
                        ps = pp.tile([2, L], F32, tag="ps", name="psg2l")
                        for seg in range(L // MM):
                            for j in range(2):
                                nc.tensor.matmul(
                                    ps[:, seg * MM:(seg + 1) * MM],
                                    lhsT=s_g2m[:, 2 * (2 * pr + j):
                                               2 * (2 * pr + j) + 2],
                                    rhs=rts[2 * pr + j][:,
                                                        seg * MM:(seg + 1) * MM],
                                    start=(j == 0), stop=(j == 1))
                        orow = mp.tile([2, L], F32, tag="orow", name="orow")
                        nc.scalar.copy(out=orow, in_=ps)
                        nc.sync.dma_start(out=d_out[2 * pr:2 * pr + 2, :],
                                          in_=orow)

    _spill_excess_waits(nc)
    return nc


def _host_prep(inputs):
    """Per-core input maps. The input projection, both time-encoder layers'
    contributions, and every bias are composed into the gate/head weights
    (fp64) so the device operates on R = [te1_hidden(64); x(2); ones(1)]."""
    f = {k: np.asarray(v, np.float64) for k, v in inputs.items()}

    def dt(a):
        return np.ascontiguousarray(a.astype(np.float32).astype(NP_DT))

    def f32c(a):
        return np.ascontiguousarray(a.astype(np.float32))

    def gate_w(pw, pb, w, b):
        """(67,256) weight in the R basis for pre = (xc@[pw;pb]) @ w + b."""
        te_part = f["te_w2"] @ pw[2:66] @ w                  # (64,256)
        x_part = pw[0:2] @ w                                 # (2,256)
        ones_row = f["te_b2"] @ pw[2:66] @ w + pb @ w + b    # (256,)
        return np.concatenate([te_part, x_part, ones_row[None, :]], axis=0)

    common = {}
    blocks = []
    for d, pw, pb in (("f", f["fproj_w"], f["fproj_b"]),
                      ("b", f["bproj_w"], f["bproj_b"])):
        blocks.append(gate_w(pw, pb, f[f"{d}wz_w"], f[f"{d}wz_b"]))
        blocks.append(gate_w(pw, pb, f[f"{d}wh_w"], f[f"{d}wh_b"]))
    # device order: [z_f | h_f | z_b | h_b]
    common["wk"] = dt(np.concatenate(
        [blocks[0], blocks[1], blocks[2], blocks[3]], axis=1))   # (67, 1024)
    g1 = f["gh_w1"][0:2 * H]                                 # (512,128)
    common["g1k"] = dt(np.concatenate(
        [g1[128 * j:128 * (j + 1)] for j in range(4)], axis=1))  # (128,512)
    g1te = f["gh_w1"][2 * H:2 * H + TE]                      # (64,128)
    common["g1te"] = dt(np.concatenate(
        [f["te_w2"] @ g1te, np.zeros((2, HH)),
         (f["te_b2"] @ g1te + f["gh_b1"])[None, :]], axis=0))
    g2m = np.zeros((HH, 2 * BS))
    for bi in range(BS):
        g2m[:, 2 * bi + bi % 2] = f["gh_w2"][:, 0]
    common["g2m"] = dt(g2m)
    common["tw"] = f32c(np.concatenate(
        [f["te_w1"].T, f["te_b1"][:, None]], axis=1))        # (64,2)
    in_maps = []
    for c in range(NCORES):
        sl = slice(BS * c, BS * (c + 1))
        m = dict(common)
        xt = np.concatenate(
            [f["x"][sl].transpose(0, 2, 1),
             np.ones((BS, 1, L))], axis=1)                   # (BS, 3, L)
        m["xT"] = dt(xt)
        m["t"] = dt(f["t"][sl, :, 0])
        in_maps.append(m)
    return in_maps, float(f["gh_b2"][0])


_CACHE = {}


def _get_nc():
    if "nc" not in _CACHE:
        _CACHE["nc"] = _build_nc()
    return _CACHE["nc"]


def kernel(**inputs):
    nc = _get_nc()
    in_maps, gh_b2 = _host_prep(inputs)
    res = run_bass_kernel_spmd(nc, in_maps, list(range(NCORES)))
    out = np.empty((B, L, 1), np.float32)
    for c in range(NCORES):
        out[BS * c:BS * (c + 1), :, 0] = res.results[c]["out"] + gh_b2
    return out


def _build_sharded_exec(nc):
    """Non-donating clone of bass2jax.run_bass_via_pjrt's multi-core path so
    the executable can be launched repeatedly for timing."""
    import jax
    import concourse.mybir as mb
    from jax.experimental.shard_map import shard_map
    from jax.sharding import Mesh, PartitionSpec
    from concourse import bass2jax

    bass2jax.install_neuronx_cc_hook()
    part_name = nc.partition_id_tensor.name if nc.partition_id_tensor else None
    in_names, out_names, out_avals, zero_outs = [], [], [], []
    for alloc in nc.m.functions[0].allocations:
        if not isinstance(alloc, mb.MemoryLocationSet):
            continue
        name = alloc.memorylocations[0].name
        if alloc.kind == "ExternalInput":
            if name != part_name:
                in_names.append(name)
        elif alloc.kind == "ExternalOutput":
            shape = tuple(alloc.tensor_shape)
            dtype = mb.dt.np(alloc.dtype)
            out_names.append(name)
            out_avals.append(jax.core.ShapedArray(shape, dtype))
            zero_outs.append(np.zeros(shape, dtype))
    n_params = len(in_names)
    all_names = in_names + out_names
    if part_name is not None:
        all_names = all_names + [part_name]

    def _body(*args):
        operands = list(args)
        if part_name is not None:
            operands.append(bass2jax.partition_id_tensor())
        outs = bass2jax._bass_exec_p.bind(
            *operands,
            out_avals=tuple(out_avals),
            in_names=tuple(all_names),
            out_names=tuple(out_names),
            lowering_input_output_aliases=(),
            sim_require_finite=True,
            sim_require_nnan=True,
            nc=nc,
        )
        return tuple(outs)

    devices = jax.devices()[:NCORES]
    mesh = Mesh(np.asarray(devices), ("core",))
    nin = n_params + len(out_names)
    sharded = jax.jit(
        shard_map(_body, mesh=mesh,
                  in_specs=(PartitionSpec("core"),) * nin,
                  out_specs=(PartitionSpec("core"),) * len(out_names),
                  check_rep=False),
        keep_unused=True,
    )
    return sharded, mesh, in_names, out_names, zero_outs


def _timed_launch(nc, in_maps, iters):
    import jax
    from jax.sharding import NamedSharding, PartitionSpec

    sharded, mesh, in_names, out_names, zero_outs = _build_sharded_exec(nc)
    sh = NamedSharding(mesh, PartitionSpec("core"))
    concat_in = [
        np.concatenate([np.asarray(in_maps[c][n]) for c in range(NCORES)], axis=0)
        for n in in_names
    ]
    concat_zero = [
        np.zeros((NCORES * z.shape[0], *z.shape[1:]), z.dtype) for z in zero_outs
    ]
    args = [jax.device_put(a, sh) for a in concat_in + concat_zero]
    out = sharded(*args)
    jax.block_until_ready(out)
    ts = []
    for _ in range(iters):
        t0 = time.perf_counter()
        out = sharded(*args)
        jax.block_until_ready(out)
        ts.append(time.perf_counter() - t0)
    return min(ts)


def bench(inputs, iters=10, r_hi=25, k=16, rounds=8):
    """On-device per-iteration kernel time (ns) via async-pipelined repeat
    slope: launch k executions back-to-back (async dispatch) of NEFFs with
    1 and r_hi repeats of the whole per-core computation; the wall-clock
    difference divided by k*(r_hi-1) removes launch/RPC overhead."""
    import jax
    in_maps, _ = _host_prep(inputs)
    ex_lo, args_lo = _prep_exec(_build_nc(repeats=1), in_maps)
    ex_hi, args_hi = _prep_exec(_build_nc(repeats=r_hi), in_maps)

    def run_async(ex, args, n):
        t0 = time.perf_counter()
        outs = [ex(*args) for _ in range(n)]
        jax.block_until_ready(outs)
        return time.perf_counter() - t0

    run_async(ex_lo, args_lo, 2)
    run_async(ex_hi, args_hi, 2)
    los, his = [], []
    for _ in range(rounds):
        los.append(run_async(ex_lo, args_lo, k))
        his.append(run_async(ex_hi, args_hi, k))
    los, his = sorted(los), sorted(his)
    med = (his[len(his) // 2] - los[len(los) // 2]) / (k * (r_hi - 1)) * 1e9
    lo = (min(his) - min(los)) / (k * (r_hi - 1)) * 1e9
    # median-based slope is the stabler statistic on this transport
    return med if med > 0 else lo


def _prep_exec(nc, in_maps):
    import jax
    from jax.sharding import NamedSharding, PartitionSpec
    sharded, mesh, in_names, out_names, zero_outs = _build_sharded_exec(nc)
    sh = NamedSharding(mesh, PartitionSpec("core"))
    concat_in = [
        np.concatenate([np.asarray(in_maps[c][n]) for c in range(NCORES)], axis=0)
        for n in in_names
    ]
    concat_zero = [
        np.zeros((NCORES * z.shape[0], *z.shape[1:]), z.dtype) for z in zero_outs
    ]
    args = [jax.device_put(a, sh) for a in concat_in + concat_zero]
    return sharded, args
